# revision 24
# baseline (speedup 1.0000x reference)
"""Trainium2 Bass kernel for masked multi-head attention (B=4, S=1024, D=1024, H=16).

Sharding: 8 cores; core c handles batch b=c//2, query rows [r*512,(r+1)*512) with
r=c%2. No collectives: K/V projection work is duplicated within each core pair
(cheaper than an all-reduce on this fabric — a quad AllGather of the projected
K/V measured ~40-85us, which would sit on the critical path). All matmuls run
as float32r (tf32) at full PE rate.

Layouts (per core), everything transposed on the host so contraction dims land on
SBUF partitions:
  xtq [D, 512]  = queries[b, rows].T          xtk/xtv [D, SK] = keys/values[b,:SK].T
  wq, wo [D, D] natural
  vmask [128, NK]: vmask[p,t] = 1.0 if t*128+p < valid_len[b] else 0.0

Pipeline (all phases overlap-scheduled):
  * wq/xtq DMAs interleaved per k-tile; Q^T projection runs k-outer with 3
    concurrent PSUM accumulators so matmuls start on the first 0.75MB of DMA
    instead of waiting for the full 6MB prefix.
  * K^T projection k-outer in (m-group, column) passes: pass 0 streams behind
    the xtk DMA. xtq/xtk/xtv share a 2-deep SBUF ring so the xtv DMA is not
    serialized behind Kproj's last read.
  * V is projected into a head-interleaved store [sk, 16*(64+1)] with a vmask
    column per head: the O^T = V_aug^T @ P^T matmul yields the attention
    output rows (0..63) AND the masked softmax denominator (row 64) in one
    accumulation. Masking is purely multiplicative via the zeroed V rows
    (exp(NEG)==0 in the reference, identical result).
  * The first 2 heads' score matmuls + exp activations are interleaved into
    the V-projection tile loop (closed single-matmul groups only — holding an
    O accumulation group open across other groups miscomputes on hardware),
    starting the Activation engine ~25us early; the attention tail is
    exp-throughput-bound on ScalarE, the overall kernel PE-bound.
  * Scores are computed transposed (S^T[sk,sq] = K_h^T-tile @ Q_h^T) with
    exp(x/8) fused on ScalarE during the PSUM->SBUF copy.
  * Softmax division: reciprocal of the denominator row on DVE, broadcast to
    64 partitions with a gpsimd partition_broadcast (SBUF->SBUF, no DRAM
    bounce), then one DVE multiply into the O^T store.
  * Output projection accumulates per m-tile; out DMAs are batched 4 m-tiles
    per descriptor to amortize DMA issue latency.
"""

import os
import numpy as np

import concourse.bass as bass
import concourse.tile as tile
from concourse import bacc, mybir
from concourse.bass_utils import run_bass_kernel_spmd

B, S, D = 4, 1024, 1024
H, HD = 16, 64
N_CORES = 8
SQ = 512  # query rows per core
F32 = mybir.dt.float32
F32R = mybir.dt.float32r
VW = 65  # per-head v_store width (64 dims + 1 mask/ones column)

_module_cache: dict[int, object] = {}


def _build_module(nk: int, reps: int = 1, phases=None, variant: int = 0):
    """Build the SPMD Bass module; nk = number of 128-row key tiles.

    reps > 1 emits the whole pipeline multiple times (same pools) — used only
    for slope-based device-time measurement in the dev harness.
    """
    def on(name):
        return phases is None or name in phases

    chunk_pt = (nk >= 7) or (variant == 1)

    sk = nk * 128
    nkt = D // 128  # contraction k-tiles for the projections
    nm = D // 128   # output row-tiles (128 douts each)

    nc = bacc.Bacc("TRN2", target_bir_lowering=False, debug=False,
                   num_devices=N_CORES)

    xtq_d = nc.dram_tensor("xtq", [D, SQ], F32, kind="ExternalInput")
    xtk_d = nc.dram_tensor("xtk", [D, sk], F32, kind="ExternalInput")
    xtv_d = nc.dram_tensor("xtv", [D, sk], F32, kind="ExternalInput")
    wq_d = nc.dram_tensor("wq", [D, D], F32, kind="ExternalInput")
    wo_d = nc.dram_tensor("wo", [D, D], F32, kind="ExternalInput")
    vm_d = nc.dram_tensor("vmask", [128, nk], F32, kind="ExternalInput")
    out_d = nc.dram_tensor("outT", [D, SQ], F32, kind="ExternalOutput")

    with tile.TileContext(nc) as tc:
        with (
            tc.tile_pool(name="w", bufs=1) as wpool,
            tc.tile_pool(name="xtq", bufs=1) as _xtqpool,
            tc.tile_pool(name="xtkv", bufs=2) as xtkvpool,
            tc.tile_pool(name="qt", bufs=1) as qtpool,
            tc.tile_pool(name="kt", bufs=1) as ktpool,
            tc.tile_pool(name="vs", bufs=1) as vspool,
            tc.tile_pool(name="pt", bufs=(4 if ((nk >= 7) or (variant == 1)) else max(2, int(os.environ.get("K_EARLY", "2"))))) as ptpool,
            tc.tile_pool(name="ot", bufs=1) as otpool,
            tc.tile_pool(name="small", bufs=1) as smallpool,
            tc.tile_pool(name="inv", bufs=int(os.environ.get("K_INV", "2"))) as invpool,
            tc.tile_pool(name="osb", bufs=int(os.environ.get("K_OSB", "1"))) as osbpool,
            tc.tile_pool(name="psA", bufs=int(os.environ.get("K_PSA", "3")), space="PSUM") as psA,
            tc.tile_pool(name="psS", bufs=int(os.environ.get("K_PSS", "2")), space="PSUM") as psS,
            tc.tile_pool(name="psO", bufs=int(os.environ.get("K_PSO", "3")), space="PSUM") as psO,
        ):
          for _rep in range(reps):
              # ---- resident weights (wq slot later reused for wo via same tag)
              # wq / xtq DMAs interleaved per k-tile so the k-outer Q
              # projection can start after the first pair lands.
              wq_sb = wpool.tile([128, nkt * D], F32R, tag="w")
              xtq_sb = xtkvpool.tile([128, nkt * SQ], F32R, tag="xtkv")
              for k in range(nkt):
                  nc.sync.dma_start(out=wq_sb[:, k * D:(k + 1) * D],
                                    in_=wq_d.ap()[k * 128:(k + 1) * 128, :].bitcast(F32R))
                  nc.sync.dma_start(out=xtq_sb[:, k * SQ:(k + 1) * SQ],
                                    in_=xtq_d.ap()[k * 128:(k + 1) * 128, :].bitcast(F32R))

              vmask_sb = smallpool.tile([128, nk], F32, tag="vmask")
              nc.sync.dma_start(out=vmask_sb[:], in_=vm_d.ap())
              ones16 = smallpool.tile([128, 16], F32, tag="ones16")
              nc.vector.memset(ones16[:], 1.0)

              xtk_sb = xtkvpool.tile([128, nkt * sk], F32R, tag="xtkv")
              for k in range(nkt):
                  nc.sync.dma_start(out=xtk_sb[:, k * sk:(k + 1) * sk],
                                    in_=xtk_d.ap()[k * 128:(k + 1) * 128, :].bitcast(F32R))
              xtv_sb = xtkvpool.tile([128, nkt * sk], F32R, tag="xtkv")
              for k in range(nkt):
                  nc.sync.dma_start(out=xtv_sb[:, k * sk:(k + 1) * sk],
                                    in_=xtv_d.ap()[k * 128:(k + 1) * 128, :].bitcast(F32R))

              # ---- Q^T projection: qt[dout, sq]; k-outer with 4 PSUM
              # accumulators so matmuls start on the first wq/xtq k-tile.
              qt_sb = qtpool.tile([128, nm * SQ], F32R, tag="qt")
              gw = 3 if os.environ.get("K_KOUTER", "1") == "1" else 1
              mgroups = [list(range(o, min(o + gw, nm))) for o in range(0, nm, gw)]
              for ms in (mgroups if on("qt") else []):
                  pss = {m: psA.tile([128, SQ], F32, tag="proj", name=f"qa{m}") for m in ms}
                  for k in range(nkt):
                      for m in ms:
                          nc.tensor.matmul(
                              pss[m][:],
                              wq_sb[:, k * D + m * 128: k * D + (m + 1) * 128],
                              xtq_sb[:, k * SQ:(k + 1) * SQ],
                              start=(k == 0), stop=(k == nkt - 1))
                  for m in ms:
                      nc.vector.tensor_copy(qt_sb[:, m * SQ:(m + 1) * SQ], pss[m][:])

              # ---- K^T projection: kt[dout, sk]; k-outer in (m-group, col)
              # passes so pass 0 streams behind the xtk DMA.
              kt_sb = ktpool.tile([128, nm * sk], F32R, tag="kt")
              nsplits = [(o, min(512, sk - o)) for o in range(0, sk, 512)]
              for (noff, nw) in (nsplits if on("kt") else []):
                  for ms in mgroups:
                      pss = {m: psA.tile([128, 512], F32, tag="proj", name=f"ka{m}") for m in ms}
                      for k in range(nkt):
                          for m in ms:
                              nc.tensor.matmul(
                                  pss[m][:, :nw],
                                  wq_sb[:, k * D + m * 128: k * D + (m + 1) * 128],
                                  xtk_sb[:, k * sk + noff: k * sk + noff + nw],
                                  start=(k == 0), stop=(k == nkt - 1))
                      for m in ms:
                          nc.vector.tensor_copy(
                              kt_sb[:, m * sk + noff: m * sk + noff + nw],
                              pss[m][:, :nw])

              # wo loads into the wq slot; Tile serializes on wq's last reader
              wo_sb = wpool.tile([128, nkt * D], F32R, tag="w")
              for k in range(nkt):
                  nc.sync.dma_start(out=wo_sb[:, k * D:(k + 1) * D],
                                    in_=wo_d.ap()[k * 128:(k + 1) * 128, :].bitcast(F32R))

              # ---- V projection into head-interleaved store with mask
              # columns; heads 0/1's score/exp/O work is interleaved into the
              # tile loop so the Activation engine starts ~25us earlier.
              ot_sb = otpool.tile([128, nm * SQ], F32R, tag="ot")
              n_early = int(os.environ.get("K_EARLY", "2")) if not chunk_pt else 0
              heads_early = tuple(range(n_early)) if (on("v") and on("attn")) else ()
              pt_early = {h: ptpool.tile([128, nk * SQ], F32R, tag="pt",
                                         name=f"pte{h}") for h in heads_early}
              vs_sb = vspool.tile([128, nk * H * VW], F32R, tag="vs")
              for t in range(nk if on("v") else 0):
                  for half in range(2):  # d columns [half*512, half*512+512)
                      ps = psA.tile([128, 512], F32, tag="proj")
                      for k in range(nkt):
                          nc.tensor.matmul(
                              ps[:],
                              xtv_sb[:, k * sk + t * 128: k * sk + (t + 1) * 128],
                              wq_sb[:, k * D + half * 512: k * D + half * 512 + 512],
                              start=(k == 0), stop=(k == nkt - 1))
                      dst = vs_sb[:, t * H * VW + half * 8 * VW:
                                  t * H * VW + (half + 1) * 8 * VW]
                      dst = dst.rearrange("p (h c) -> p h c", c=VW)[:, :, 0:HD]
                      src = ps[:].rearrange("p (h c) -> p h c", c=HD)
                      nc.vector.tensor_scalar_mul(dst, src, vmask_sb[:, t:t + 1])
                      mcols = vs_sb[:, t * H * VW: (t + 1) * H * VW]
                      mcols = mcols.rearrange("p (h c) -> p h c", c=VW)
                      mcols = mcols[:, half * 8:(half + 1) * 8, HD:VW]
                      o16 = ones16[:].rearrange("p (h o) -> p h o", o=1)
                      nc.vector.tensor_scalar_mul(
                          mcols, o16[:, half * 8:(half + 1) * 8, :],
                          vmask_sb[:, t:t + 1])
                      if half == 0:
                          for h in heads_early:
                              po = 64 * (h % 2)
                              mb = h // 2
                              ss = psS.tile([128, SQ], F32, tag="s")
                              nc.tensor.matmul(
                                  ss[:],
                                  kt_sb[po:po + 64, mb * sk + t * 128:
                                        mb * sk + (t + 1) * 128],
                                  qt_sb[po:po + 64, mb * SQ:(mb + 1) * SQ],
                                  start=True, stop=True)
                              nc.scalar.activation(
                                  pt_early[h][:, t * SQ:(t + 1) * SQ], ss[:],
                                  mybir.ActivationFunctionType.Exp, scale=0.125)

              for h in heads_early:
                  po = 64 * (h % 2)
                  mb = h // 2
                  po_ps = psO.tile([VW, SQ], F32, tag="o")
                  for t in range(nk):
                      nc.tensor.matmul(
                          po_ps[:],
                          vs_sb[:, t * H * VW + h * VW: t * H * VW + (h + 1) * VW],
                          pt_early[h][:, t * SQ:(t + 1) * SQ],
                          start=(t == 0), stop=(t == nk - 1))
                  inv = invpool.tile([1, SQ], F32, tag="inv")
                  nc.vector.reciprocal(inv[:], po_ps[64:65, :])
                  inv_rep = invpool.tile([64, SQ], F32, tag="invrep")
                  nc.gpsimd.partition_broadcast(inv_rep[:], inv[:])
                  nc.vector.tensor_mul(
                      ot_sb[po:po + 64, mb * SQ:(mb + 1) * SQ],
                      po_ps[0:64, :], inv_rep[:])

              # ---- attention for the remaining heads
              for h in range(len(heads_early) if on("attn") else H,
                             H if on("attn") else 0):
                  po = 64 * (h % 2)       # partition offset of this head's douts
                  mb = h // 2             # dout row-tile holding this head
                  if not chunk_pt:
                      # P^T per head resident; score/exp pass then O^T pass
                      pt = ptpool.tile([128, nk * SQ], F32R, tag="pt")
                      for t in range(nk):
                          ss = psS.tile([128, SQ], F32, tag="s")
                          nc.tensor.matmul(
                              ss[:],
                              kt_sb[po:po + 64, mb * sk + t * 128: mb * sk + (t + 1) * 128],
                              qt_sb[po:po + 64, mb * SQ:(mb + 1) * SQ],
                              start=True, stop=True)
                          nc.scalar.activation(pt[:, t * SQ:(t + 1) * SQ], ss[:],
                                               mybir.ActivationFunctionType.Exp,
                                               scale=0.125)
                      po_ps = psO.tile([VW, SQ], F32, tag="o")
                      for t in range(nk):
                          nc.tensor.matmul(
                              po_ps[:],
                              vs_sb[:, t * H * VW + h * VW: t * H * VW + (h + 1) * VW],
                              pt[:, t * SQ:(t + 1) * SQ],
                              start=(t == 0), stop=(t == nk - 1))
                  else:
                      # chunked P^T (smaller SBUF footprint for large nk)
                      po_ps = psO.tile([VW, SQ], F32, tag="o")
                      for t in range(nk):
                          ss = psS.tile([128, SQ], F32, tag="s")
                          nc.tensor.matmul(
                              ss[:],
                              kt_sb[po:po + 64, mb * sk + t * 128: mb * sk + (t + 1) * 128],
                              qt_sb[po:po + 64, mb * SQ:(mb + 1) * SQ],
                              start=True, stop=True)
                          ptc = ptpool.tile([128, SQ], F32R, tag="pt")
                          nc.scalar.activation(ptc[:], ss[:],
                                               mybir.ActivationFunctionType.Exp,
                                               scale=0.125)
                          nc.tensor.matmul(
                              po_ps[:],
                              vs_sb[:, t * H * VW + h * VW: t * H * VW + (h + 1) * VW],
                              ptc[:],
                              start=(t == 0), stop=(t == nk - 1),
                              skip_group_check=True)
                  inv = invpool.tile([1, SQ], F32, tag="inv")
                  nc.vector.reciprocal(inv[:], po_ps[64:65, :])
                  # broadcast inv to 64 partitions on the (idle) gpsimd engine
                  inv_rep = invpool.tile([64, SQ], F32, tag="invrep")
                  nc.gpsimd.partition_broadcast(inv_rep[:], inv[:])
                  nc.vector.tensor_mul(
                      ot_sb[po:po + 64, mb * SQ:(mb + 1) * SQ],
                      po_ps[0:64, :], inv_rep[:])

              # ---- output projection: outT[dout, sq] = Wo^T-tiles @ O^T
              # out DMAs batched 4 m-tiles at a time to amortize issue latency
              osb4 = None
              for m in range(nm if on("out") else 0):
                  ps = psA.tile([128, SQ], F32, tag="proj")
                  for k in range(nkt):
                      nc.tensor.matmul(
                          ps[:],
                          wo_sb[:, k * D + m * 128: k * D + (m + 1) * 128],
                          ot_sb[:, k * SQ:(k + 1) * SQ],
                          start=(k == 0), stop=(k == nkt - 1))
                  if m % 4 == 0:
                      osb4 = osbpool.tile([128, 4 * SQ], F32, tag="outsb")
                  nc.vector.tensor_copy(
                      osb4[:, (m % 4) * SQ:(m % 4 + 1) * SQ], ps[:])
                  if m % 4 == 3:
                      dst = out_d.ap()[(m - 3) * 128:(m + 1) * 128, :]
                      dst = dst.rearrange("(mm p) c -> p mm c", p=128)
                      nc.sync.dma_start(out=dst, in_=osb4[:])

    nc.compile()
    return nc


def kernel(queries, keys, values, valid_lengths, W_q, W_o):
    queries = np.ascontiguousarray(np.asarray(queries, dtype=np.float32))
    keys = np.ascontiguousarray(np.asarray(keys, dtype=np.float32))
    values = np.ascontiguousarray(np.asarray(values, dtype=np.float32))
    W_q = np.ascontiguousarray(np.asarray(W_q, dtype=np.float32))
    W_o = np.ascontiguousarray(np.asarray(W_o, dtype=np.float32))
    vls = np.asarray(valid_lengths).astype(np.int64)

    nk = max(1, int(-(-int(vls.max()) // 128)))  # ceil(max_vl/128)
    sk = nk * 128

    nc = _module_cache.get(nk)
    if nc is None:
        nc = _build_module(nk)
        _module_cache[nk] = nc

    in_maps = []
    for c in range(N_CORES):
        b, r = c // 2, c % 2
        vl = int(vls[b])
        vm = (np.arange(sk) < vl).astype(np.float32).reshape(nk, 128).T
        in_maps.append({
            "xtq": np.ascontiguousarray(queries[b, r * SQ:(r + 1) * SQ, :].T),
            "xtk": np.ascontiguousarray(keys[b, :sk, :].T),
            "xtv": np.ascontiguousarray(values[b, :sk, :].T),
            "wq": W_q,
            "wo": W_o,
            "vmask": np.ascontiguousarray(vm),
        })

    res = run_bass_kernel_spmd(nc, in_maps, list(range(N_CORES)))

    out = np.empty((B, S, D), dtype=np.float32)
    for c in range(N_CORES):
        b, r = c // 2, c % 2
        out[b, r * SQ:(r + 1) * SQ, :] = res.results[c]["outT"].T
    return out

